# revision 17
# baseline (speedup 1.0000x reference)
"""AttnBlock1D (BN + single-head 1x1-conv attention + residual) on 8 TRN2 cores.

Contract: kernel(**inputs) takes the FULL inputs from setup_inputs() and
returns the FULL output [4, 256, 4096] f32.

Sharding: 8 cores = 4 samples x 2 query-halves. Core i handles sample
b = i // 2 and queries [qh*2048, (qh+1)*2048) with qh = i % 2. The host
rolls x[b] along L so each core's queries are the FIRST 2048 columns --
attention is permutation-invariant over keys, so k/v built from the rolled
layout give identical softmax results. This keeps the SPMD program free of
per-core constants.

BatchNorm stats are computed locally on every core -- no collective. Any
cross-core sync would put the NEFF start skew across the 8 cores (33-65us
measured, run-variable) onto the measured span. Own-sample sums come from
Scalar-engine accumulate activations over the fp32 x tiles (the Copy pass
doubles as the bf16 cast of x); the other three samples stream in as an
fp16 copy through DVE bn_stats, quarter-tile at a time so compute chases
the DMA. fp16 quantization perturbs the batch stats by ~1e-5 relative.

The BN affine (h = x*a + d) is folded into the projections on-device:
wq_eff = wq * a (per input channel), bias_eff = w @ d + b, so the
projection matmuls read the bf16 cast of x directly and the only
stats-dependent serial work is scaling four 256x256 weight tiles.
The v-path constant (wv @ d + bv) is softmax-invariant and folds into the
output projection bias: bpe = bp + wp @ bv (host) + wp @ (wv @ d) (device).

Matmul dtype is bf16 (1 cycle/row on the PE; fp16 measured at 2
cycles/row in-kernel). PSUM accumulation is fp32. Attention scores are
computed transposed (ST[j, i] = sum_c k[c, j] q[c, i]) so the
probabilities land with j (keys) on the partition axis, which the AV
matmul contracts natively; softmax needs no max-subtraction (scores ~
N(0, 1), exp is safe in fp32) and the denominator comes from a
ones[128,128] matmul that also broadcasts it across partitions.
"""

import os

import numpy as np
import ml_dtypes

import concourse.bass as bass
import concourse.mybir as mybir
import concourse.tile as tile
from concourse import bacc
from concourse import bass_utils

F32 = mybir.dt.float32
BF16 = mybir.dt.bfloat16
F16 = mybir.dt.float16

N_CORES = 8
B, C, L = 4, 256, 4096
M = L // 2          # queries per core
EPS = 1e-5
SCALE = 1.0 / 16.0  # C ** -0.5

NCHUNK = 4          # query chunks per core
CH = M // NCHUNK    # 512 queries per chunk
NJT = L // 128      # 32 key tiles
AF = mybir.ActivationFunctionType

LAST_EXEC_NS = None
_COMPILED = None


def _build():
    nc = bacc.Bacc("TRN2", target_bir_lowering=False, debug=False,
                   num_devices=N_CORES)

    x_d = nc.dram_tensor("x", [C, L], F32, kind="ExternalInput")
    x16_d = nc.dram_tensor("x16", [C, L], BF16, kind="ExternalInput")
    xs_d = nc.dram_tensor("xs", [(B - 1) * C, L], F16, kind="ExternalInput")
    wq_d = nc.dram_tensor("wqT", [C, C], BF16, kind="ExternalInput")
    wk_d = nc.dram_tensor("wkT", [C, C], BF16, kind="ExternalInput")
    wv_d = nc.dram_tensor("wvT", [C, C], BF16, kind="ExternalInput")
    wp_d = nc.dram_tensor("wpT", [C, C], BF16, kind="ExternalInput")
    bq_d = nc.dram_tensor("bq", [C, 1], F32, kind="ExternalInput")
    bk_d = nc.dram_tensor("bk", [C, 1], F32, kind="ExternalInput")
    bp_d = nc.dram_tensor("bpe", [C, 1], F32, kind="ExternalInput")
    gam_d = nc.dram_tensor("gamma", [C, 1], F32, kind="ExternalInput")
    bet_d = nc.dram_tensor("beta", [C, 1], F32, kind="ExternalInput")
    out_d = nc.dram_tensor("out", [C, M], F32, kind="ExternalOutput")

    with tile.TileContext(nc) as tc:
        with (
            tc.tile_pool(name="big", bufs=1) as big,
            tc.tile_pool(name="pt", bufs=2) as ptp,
            tc.tile_pool(name="small", bufs=2) as sm,
            tc.tile_pool(name="eps", bufs=2) as epi,
            tc.tile_pool(name="ps_s", bufs=2, space="PSUM") as ps_s,
            tc.tile_pool(name="ps_acc", bufs=1, space="PSUM") as ps_acc,
            tc.tile_pool(name="ps_o", bufs=1, space="PSUM") as ps_o,
        ):
            # ---- DMA: x16 first (stats+compute), xs, weights; f32 x last
            x16_t = [big.tile([128, L], BF16, name=f"x16_{h}")
                     for h in range(2)]
            for h in range(2):
                nc.sync.dma_start(x16_t[h][:], x16_d[h * 128:(h + 1) * 128, :])

            vecs = {}
            for nm, d in (("bq", bq_d), ("bk", bk_d), ("bpe", bp_d),
                          ("gam", gam_d), ("bet", bet_d)):
                vecs[nm] = [big.tile([128, 1], F32, name=f"{nm}{h}")
                            for h in range(2)]
                for h in range(2):
                    nc.sync.dma_start(vecs[nm][h][:],
                                      d[h * 128:(h + 1) * 128, :])

            # ------- BN stats, split across DVE (bn_stats) and ACT --------
            # 8 stat tiles per core: own sample (x16, bf16) + 3 other
            # samples (xs, fp16), each [128, 4096] per channel-half.
            # Per tile: chunks 0-4 go through DVE bn_stats, chunks 5-7
            # through ACT Square/Copy accumulations -- balances the two
            # engines so stats finish right behind the DMA stream.
            s6_dve = [sm.tile([128, 4 * 5 * 6], F32, name=f"s6d{h}")
                      for h in range(2)]
            asum = [sm.tile([128, 12], F32, name=f"asum{h}") for h in range(2)]
            assq = [sm.tile([128, 12], F32, name=f"assq{h}") for h in range(2)]

            def stat_tile(tile_ap, h, tidx):
                for i in range(5):
                    nc.vector.bn_stats(
                        s6_dve[h][:, (tidx * 5 + i) * 6:(tidx * 5 + i + 1) * 6],
                        tile_ap[:, i * 512:(i + 1) * 512])
                for ji, i in enumerate(range(5, 8)):
                    col = tidx * 3 + ji
                    cs = slice(i * 512, (i + 1) * 512)
                    scr0 = sm.tile([128, 512], BF16, tag="scr", bufs=2,
                                   name=f"scrS{h}_{tidx}_{i}")
                    nc.scalar.activation(scr0[:], tile_ap[:, cs], AF.Copy,
                                         accum_out=asum[h][:, col:col + 1])
                    scr1 = sm.tile([128, 512], BF16, tag="scr", bufs=2,
                                   name=f"scrQ{h}_{tidx}_{i}")
                    nc.scalar.activation(scr1[:], tile_ap[:, cs], AF.Square,
                                         accum_out=assq[h][:, col:col + 1])

            for h in range(2):
                stat_tile(x16_t[h][:], h, 0)

            for s in range(B - 1):
                for h in range(2):
                    xs_t = sm.tile([128, L], F16, tag="xs", bufs=2,
                                   name=f"xs{s}_{h}")
                    row0 = s * C + h * 128
                    for q2 in range(2):
                        qs = slice(q2 * 2048, (q2 + 1) * 2048)
                        nc.sync.dma_start(xs_t[:, qs],
                                          xs_d[row0:row0 + 128, qs])
                    stat_tile(xs_t[:], h, 1 + s)

            # weights stream in behind the stats inputs
            w_t = {}
            for nm, d in (("q", wq_d), ("k", wk_d), ("v", wv_d), ("p", wp_d)):
                w_t[nm] = [big.tile([128, C], BF16, name=f"w{nm}{h}")
                           for h in range(2)]
                for h in range(2):
                    nc.sync.dma_start(w_t[nm][h][:],
                                      d[h * 128:(h + 1) * 128, :])

            # f32 x arrives late; only the epilogue residual reads it
            x_t = [big.tile([128, L], F32, name=f"x{h}") for h in range(2)]
            for h in range(2):
                nc.sync.dma_start(x_t[h][:], x_d[h * 128:(h + 1) * 128, :])

            # ------- combine stats -> a (scale), d (shift) per channel ----
            ND = 4 * 5 * 512          # elements covered by the DVE packs
            NT = B * L
            a_t, d_t = [], []
            for h in range(2):
                s2 = sm.tile([128, 2], F32, name=f"s2_{h}")
                nc.vector.bn_aggr(s2[:], s6_dve[h][:])
                sa = sm.tile([128, 1], F32, name=f"sa{h}")
                nc.vector.reduce_sum(sa[:], asum[h][:], axis=mybir.AxisListType.X)
                qa = sm.tile([128, 1], F32, name=f"qa{h}")
                nc.vector.reduce_sum(qa[:], assq[h][:], axis=mybir.AxisListType.X)
                tot = sm.tile([128, 1], F32, name=f"tot{h}")
                nc.vector.scalar_tensor_tensor(
                    out=tot[:], in0=s2[:, 0:1], scalar=float(ND), in1=sa[:],
                    op0=mybir.AluOpType.mult, op1=mybir.AluOpType.add)
                mo2 = sm.tile([128, 1], F32, name=f"mo2{h}")
                nc.vector.tensor_mul(mo2[:], s2[:, 0:1], s2[:, 0:1])
                e2o = sm.tile([128, 1], F32, name=f"e2o{h}")
                nc.vector.tensor_add(e2o[:], s2[:, 1:2], mo2[:])
                totq = sm.tile([128, 1], F32, name=f"totq{h}")
                nc.vector.scalar_tensor_tensor(
                    out=totq[:], in0=e2o[:], scalar=float(ND), in1=qa[:],
                    op0=mybir.AluOpType.mult, op1=mybir.AluOpType.add)
                gm = sm.tile([128, 1], F32, name=f"gm{h}")
                nc.vector.tensor_scalar_mul(gm[:], tot[:], 1.0 / NT)
                ge2 = sm.tile([128, 1], F32, name=f"ge2{h}")
                nc.vector.tensor_scalar_mul(ge2[:], totq[:], 1.0 / NT)
                mm_ = sm.tile([128, 1], F32, name=f"mm{h}")
                nc.vector.tensor_mul(mm_[:], gm[:], gm[:])
                var = sm.tile([128, 1], F32, name=f"var{h}")
                nc.vector.tensor_sub(var[:], ge2[:], mm_[:])
                nc.vector.tensor_scalar_add(var[:], var[:], EPS)
                sd = sm.tile([128, 1], F32, name=f"sd{h}")
                nc.scalar.activation(sd[:], var[:], AF.Sqrt)
                rs = sm.tile([128, 1], F32, name=f"rs{h}")
                nc.vector.reciprocal(rs[:], sd[:])
                a = sm.tile([128, 1], F32, name=f"a{h}")
                nc.vector.tensor_mul(a[:], rs[:], vecs["gam"][h][:])
                ma = sm.tile([128, 1], F32, name=f"ma{h}")
                nc.vector.tensor_mul(ma[:], gm[:], a[:])
                dd = sm.tile([128, 1], F32, name=f"d{h}")
                nc.vector.tensor_sub(dd[:], vecs["bet"][h][:], ma[:])
                a_t.append(a)
                d_t.append(dd)

            # ------- fold BN affine into weights + effective biases -------
            # b*_eff = w @ d + b uses the RAW weights (tiny matvecs), then
            # w is scaled IN PLACE: w[c, o] *= a[c].
            # d as a bf16 [128,1] for the tiny matvecs
            d16 = [sm.tile([128, 1], BF16, name=f"d16_{h}") for h in range(2)]
            for h in range(2):
                nc.vector.tensor_copy(d16[h][:], d_t[h][:])

            def matvec(wtiles, rhs16, name):
                """out[o] = sum_c w[o, c] * rhs[c] as [2][128, 1] sbuf f32"""
                outs = []
                for oh in range(2):
                    ps = ps_s.tile([128, 1], F32, tag="s", name=f"mv_{name}{oh}")
                    for ch in range(2):
                        nc.tensor.matmul(
                            ps[:],
                            wtiles[ch][:, oh * 128:(oh + 1) * 128],
                            rhs16[ch][:],
                            start=(ch == 0), stop=(ch == 1),
                        )
                    o = sm.tile([128, 1], F32, name=f"mvo_{name}{oh}")
                    nc.vector.tensor_copy(o[:], ps[:])
                    outs.append(o)
                return outs

            wqd = matvec(w_t["q"], d16, "q")
            wkd = matvec(w_t["k"], d16, "k")
            wvd = matvec(w_t["v"], d16, "v")
            bq_e, bk_e = [], []
            for oh in range(2):
                t = sm.tile([128, 1], F32, name=f"bqe{oh}")
                nc.vector.tensor_add(t[:], wqd[oh][:], vecs["bq"][oh][:])
                bq_e.append(t)
                t = sm.tile([128, 1], F32, name=f"bke{oh}")
                nc.vector.tensor_add(t[:], wkd[oh][:], vecs["bk"][oh][:])
                bk_e.append(t)
            # bpe_eff = bpe + wp @ (wv @ d)
            wvd16 = [sm.tile([128, 1], BF16, name=f"wvd16_{h}")
                     for h in range(2)]
            for h in range(2):
                nc.vector.tensor_copy(wvd16[h][:], wvd[h][:])
            wpwvd = matvec(w_t["p"], wvd16, "p")
            bp_e = []
            for oh in range(2):
                t = sm.tile([128, 1], F32, name=f"bpe_e{oh}")
                nc.vector.tensor_add(t[:], wpwvd[oh][:], vecs["bpe"][oh][:])
                bp_e.append(t)

            for nm in ("q", "k", "v"):
                for h in range(2):
                    nc.vector.tensor_scalar_mul(
                        w_t[nm][h][:], w_t[nm][h][:], a_t[h][:])

            # ---------------- projections (read x16 directly) -------------
            q_t = [big.tile([128, M], BF16, name=f"q{h}") for h in range(2)]
            k_t = [big.tile([128, L], BF16, name=f"k{h}") for h in range(2)]
            vT_t = big.tile([128, NJT * 256], BF16, name="vT")

            for oh in range(2):
                for it in range(M // 512):
                    ps = ps_s.tile([128, 512], F32, tag="s", name="ps_q")
                    for ch in range(2):
                        nc.tensor.matmul(
                            ps[:],
                            w_t["q"][ch][:, oh * 128:(oh + 1) * 128],
                            x16_t[ch][:, it * 512:(it + 1) * 512],
                            start=(ch == 0), stop=(ch == 1),
                        )
                    nc.vector.tensor_scalar_add(
                        q_t[oh][:, it * 512:(it + 1) * 512], ps[:],
                        bq_e[oh][:])

            for oh in range(2):
                for it in range(L // 512):
                    ps = ps_s.tile([128, 512], F32, tag="s", name="ps_k")
                    for ch in range(2):
                        nc.tensor.matmul(
                            ps[:],
                            w_t["k"][ch][:, oh * 128:(oh + 1) * 128],
                            x16_t[ch][:, it * 512:(it + 1) * 512],
                            start=(ch == 0), stop=(ch == 1),
                        )
                    nc.vector.tensor_scalar_add(
                        k_t[oh][:, it * 512:(it + 1) * 512], ps[:],
                        bk_e[oh][:])

            for lt in range(NJT):
                ps = ps_s.tile([128, 512], F32, tag="s", name="ps_v")
                for ch in range(2):
                    nc.tensor.matmul(
                        ps[:, 0:256],
                        x16_t[ch][:, lt * 128:(lt + 1) * 128],
                        w_t["v"][ch][:],
                        start=(ch == 0), stop=(ch == 1),
                    )
                nc.vector.tensor_copy(
                    vT_t[:, lt * 256:(lt + 1) * 256], ps[:, 0:256])

            ones_t = big.tile([128, 128], BF16, name="ones")
            nc.vector.memset(ones_t[:], 1.0)

            # ---------------- attention, chunk by chunk ----------------
            for cn in range(NCHUNK):
                i0 = cn * CH
                pT = ptp.tile([128, NJT * CH], BF16, tag="pT", name=f"pT{cn}")
                for jp in range(NJT // 2):
                    ps = ps_s.tile([128, 1024], F32, tag="s", name="ps_sc")
                    for half in range(2):
                        jt = jp * 2 + half
                        for ch in range(2):
                            nc.tensor.matmul(
                                ps[:, half * 512:(half + 1) * 512],
                                k_t[ch][:, jt * 128:(jt + 1) * 128],
                                q_t[ch][:, i0:i0 + CH],
                                start=(ch == 0), stop=(ch == 1),
                            )
                    nc.scalar.activation(
                        pT[:, jp * 1024:(jp + 1) * 1024], ps[:],
                        AF.Exp, scale=SCALE)

                ps_av = [ps_acc.tile([128, CH], F32, tag=f"av{ch}",
                                     name=f"av{ch}_{cn}") for ch in range(2)]
                ps_den = ps_acc.tile([128, CH], F32, tag="den",
                                     name=f"den{cn}")
                for jt in range(NJT):
                    pslice = pT[:, jt * CH:(jt + 1) * CH]
                    for ch in range(2):
                        nc.tensor.matmul(
                            ps_av[ch][:],
                            vT_t[:, jt * 256 + ch * 128:jt * 256 + (ch + 1) * 128],
                            pslice,
                            start=(jt == 0), stop=(jt == NJT - 1),
                        )
                    nc.tensor.matmul(
                        ps_den[:], ones_t[:], pslice,
                        start=(jt == 0), stop=(jt == NJT - 1),
                    )

                rec = epi.tile([128, CH], F32, tag="rec", name=f"rec{cn}")
                nc.vector.reciprocal_approx_fast(rec[:], ps_den[:])

                at_t = []
                for ch in range(2):
                    at = epi.tile([128, CH], BF16, tag=f"at{ch}",
                                  name=f"at{ch}_{cn}")
                    nc.vector.tensor_mul(at[:], ps_av[ch][:], rec[:])
                    at_t.append(at)

                for oh in range(2):
                    ps = ps_o.tile([128, CH], F32, tag="o", name=f"po{oh}_{cn}")
                    for ch in range(2):
                        nc.tensor.matmul(
                            ps[:],
                            w_t["p"][ch][:, oh * 128:(oh + 1) * 128],
                            at_t[ch][:],
                            start=(ch == 0), stop=(ch == 1),
                        )
                    res = epi.tile([128, CH], F32, tag="res",
                                   name=f"res{oh}_{cn}")
                    nc.vector.scalar_tensor_tensor(
                        out=res[:], in0=ps[:], scalar=bp_e[oh][:],
                        in1=x_t[oh][:, i0:i0 + CH],
                        op0=mybir.AluOpType.add, op1=mybir.AluOpType.add,
                    )
                    nc.sync.dma_start(
                        out_d[oh * 128:(oh + 1) * 128, i0:i0 + CH], res[:])

    nc.compile()
    return nc


def kernel(x, gamma, beta, wq, bq, wk, bk, wv, bv, wp, bp):
    global _COMPILED, LAST_EXEC_NS
    x = np.asarray(x, np.float32)
    if _COMPILED is None:
        _COMPILED = _build()
    nc = _COMPILED

    common = {
        "wqT": np.ascontiguousarray(np.asarray(wq, np.float32).T).astype(ml_dtypes.bfloat16),
        "wkT": np.ascontiguousarray(np.asarray(wk, np.float32).T).astype(ml_dtypes.bfloat16),
        "wvT": np.ascontiguousarray(np.asarray(wv, np.float32).T).astype(ml_dtypes.bfloat16),
        "wpT": np.ascontiguousarray(np.asarray(wp, np.float32).T).astype(ml_dtypes.bfloat16),
        "bq": np.asarray(bq, np.float32).reshape(C, 1),
        "bk": np.asarray(bk, np.float32).reshape(C, 1),
        "bpe": (np.asarray(bp, np.float32)
                + np.asarray(wp, np.float32) @ np.asarray(bv, np.float32)
                ).reshape(C, 1),
        "gamma": np.asarray(gamma, np.float32).reshape(C, 1),
        "beta": np.asarray(beta, np.float32).reshape(C, 1),
    }

    x16 = [np.ascontiguousarray(x[b]).astype(np.float16) for b in range(B)]

    in_maps = []
    for core in range(N_CORES):
        b, qh = core // 2, core % 2
        xb = x[b]
        if qh:
            xb = np.ascontiguousarray(np.roll(xb, -M, axis=1))
        others = np.concatenate([x16[s] for s in range(B) if s != b])
        in_maps.append({"x": xb, "x16": xb.astype(ml_dtypes.bfloat16),
                        "xs": others, **common})

    trace = os.environ.get("BASS_KERNEL_TRACE", "") == "1"
    res = bass_utils.run_bass_kernel_spmd(
        nc, in_maps, core_ids=list(range(N_CORES)), trace=trace)
    LAST_EXEC_NS = res.exec_time_ns

    out = np.empty((B, C, L), np.float32)
    for core in range(N_CORES):
        b, qh = core // 2, core % 2
        out[b, :, qh * M:(qh + 1) * M] = res.results[core]["out"]
    return out


# revision 18
# speedup vs baseline: 1.0171x; 1.0171x over previous
"""AttnBlock1D (BN + single-head 1x1-conv attention + residual) on 8 TRN2 cores.

Contract: kernel(**inputs) takes the FULL inputs from setup_inputs() and
returns the FULL output [4, 256, 4096] f32.

Sharding: 8 cores = 4 samples x 2 query-halves. Core i handles sample
b = i // 2 and queries [qh*2048, (qh+1)*2048) with qh = i % 2. The host
rolls x[b] along L so each core's queries are the FIRST 2048 columns --
attention is permutation-invariant over keys, so k/v built from the rolled
layout give identical softmax results. This keeps the SPMD program free of
per-core constants.

BatchNorm stats are computed locally on every core -- no collective. Any
cross-core sync would put the NEFF start skew across the 8 cores (33-65us
measured, run-variable) onto the measured span. Own-sample sums come from
Scalar-engine accumulate activations over the fp32 x tiles (the Copy pass
doubles as the bf16 cast of x); the other three samples stream in as an
fp16 copy through DVE bn_stats, quarter-tile at a time so compute chases
the DMA. fp16 quantization perturbs the batch stats by ~1e-5 relative.

The BN affine (h = x*a + d) is folded into the projections on-device:
wq_eff = wq * a (per input channel), bias_eff = w @ d + b, so the
projection matmuls read the bf16 cast of x directly and the only
stats-dependent serial work is scaling four 256x256 weight tiles.
The v-path constant (wv @ d + bv) is softmax-invariant and folds into the
output projection bias: bpe = bp + wp @ bv (host) + wp @ (wv @ d) (device).

Matmul dtype is bf16 (1 cycle/row on the PE; fp16 measured at 2
cycles/row in-kernel). PSUM accumulation is fp32. Attention scores are
computed transposed (ST[j, i] = sum_c k[c, j] q[c, i]) so the
probabilities land with j (keys) on the partition axis, which the AV
matmul contracts natively; softmax needs no max-subtraction (scores ~
N(0, 1), exp is safe in fp32) and the denominator comes from a
ones[128,128] matmul that also broadcasts it across partitions.
"""

import os

import numpy as np
import ml_dtypes

import concourse.bass as bass
import concourse.mybir as mybir
import concourse.tile as tile
from concourse import bacc
from concourse import bass_utils

F32 = mybir.dt.float32
BF16 = mybir.dt.bfloat16
F16 = mybir.dt.float16

N_CORES = 8
B, C, L = 4, 256, 4096
M = L // 2          # queries per core
EPS = 1e-5
SCALE = 1.0 / 16.0  # C ** -0.5

NCHUNK = 4          # query chunks per core
CH = M // NCHUNK    # 512 queries per chunk
NJT = L // 128      # 32 key tiles
AF = mybir.ActivationFunctionType

LAST_EXEC_NS = None
_COMPILED = None


def _build():
    nc = bacc.Bacc("TRN2", target_bir_lowering=False, debug=False,
                   num_devices=N_CORES)

    x_d = nc.dram_tensor("x", [C, L], F32, kind="ExternalInput")
    x16_d = nc.dram_tensor("x16", [C, L], BF16, kind="ExternalInput")
    xs_d = nc.dram_tensor("xs", [(B - 1) * C, L], mybir.dt.float8e4, kind="ExternalInput")
    wq_d = nc.dram_tensor("wqT", [C, C], BF16, kind="ExternalInput")
    wk_d = nc.dram_tensor("wkT", [C, C], BF16, kind="ExternalInput")
    wv_d = nc.dram_tensor("wvT", [C, C], BF16, kind="ExternalInput")
    wp_d = nc.dram_tensor("wpT", [C, C], BF16, kind="ExternalInput")
    bq_d = nc.dram_tensor("bq", [C, 1], F32, kind="ExternalInput")
    bk_d = nc.dram_tensor("bk", [C, 1], F32, kind="ExternalInput")
    bp_d = nc.dram_tensor("bpe", [C, 1], F32, kind="ExternalInput")
    gam_d = nc.dram_tensor("gamma", [C, 1], F32, kind="ExternalInput")
    bet_d = nc.dram_tensor("beta", [C, 1], F32, kind="ExternalInput")
    out_d = nc.dram_tensor("out", [C, M], F32, kind="ExternalOutput")

    with tile.TileContext(nc) as tc:
        with (
            tc.tile_pool(name="big", bufs=1) as big,
            tc.tile_pool(name="pt", bufs=2) as ptp,
            tc.tile_pool(name="small", bufs=2) as sm,
            tc.tile_pool(name="eps", bufs=2) as epi,
            tc.tile_pool(name="ps_s", bufs=2, space="PSUM") as ps_s,
            tc.tile_pool(name="ps_acc", bufs=1, space="PSUM") as ps_acc,
            tc.tile_pool(name="ps_o", bufs=1, space="PSUM") as ps_o,
        ):
            # ---- DMA: x16 first (stats+compute), xs, weights; f32 x last
            x16_t = [big.tile([128, L], BF16, name=f"x16_{h}")
                     for h in range(2)]
            for h in range(2):
                nc.sync.dma_start(x16_t[h][:], x16_d[h * 128:(h + 1) * 128, :])

            vecs = {}
            for nm, d in (("bq", bq_d), ("bk", bk_d), ("bpe", bp_d),
                          ("gam", gam_d), ("bet", bet_d)):
                vecs[nm] = [big.tile([128, 1], F32, name=f"{nm}{h}")
                            for h in range(2)]
                for h in range(2):
                    nc.sync.dma_start(vecs[nm][h][:],
                                      d[h * 128:(h + 1) * 128, :])

            # ------- BN stats, split across DVE (bn_stats) and ACT --------
            # 8 stat tiles per core: own sample (x16, bf16) + 3 other
            # samples (xs, fp16), each [128, 4096] per channel-half.
            # Per tile: chunks 0-4 go through DVE bn_stats, chunks 5-7
            # through ACT Square/Copy accumulations -- balances the two
            # engines so stats finish right behind the DMA stream.
            s6_dve = [sm.tile([128, 4 * 5 * 6], F32, name=f"s6d{h}")
                      for h in range(2)]
            asum = [sm.tile([128, 12], F32, name=f"asum{h}") for h in range(2)]
            assq = [sm.tile([128, 12], F32, name=f"assq{h}") for h in range(2)]

            def stat_tile(tile_ap, h, tidx):
                for i in range(5):
                    nc.vector.bn_stats(
                        s6_dve[h][:, (tidx * 5 + i) * 6:(tidx * 5 + i + 1) * 6],
                        tile_ap[:, i * 512:(i + 1) * 512])
                for ji, i in enumerate(range(5, 8)):
                    col = tidx * 3 + ji
                    cs = slice(i * 512, (i + 1) * 512)
                    scr0 = sm.tile([128, 512], BF16, tag="scr", bufs=2,
                                   name=f"scrS{h}_{tidx}_{i}")
                    nc.scalar.activation(scr0[:], tile_ap[:, cs], AF.Copy,
                                         accum_out=asum[h][:, col:col + 1])
                    scr1 = sm.tile([128, 512], BF16, tag="scr", bufs=2,
                                   name=f"scrQ{h}_{tidx}_{i}")
                    nc.scalar.activation(scr1[:], tile_ap[:, cs], AF.Square,
                                         accum_out=assq[h][:, col:col + 1])

            for h in range(2):
                stat_tile(x16_t[h][:], h, 0)

            for s in range(B - 1):
                for h in range(2):
                    xs_t = sm.tile([128, L], mybir.dt.float8e4, tag="xs", bufs=2,
                                   name=f"xs{s}_{h}")
                    row0 = s * C + h * 128
                    for q2 in range(2):
                        qs = slice(q2 * 2048, (q2 + 1) * 2048)
                        nc.sync.dma_start(xs_t[:, qs],
                                          xs_d[row0:row0 + 128, qs])
                    stat_tile(xs_t[:], h, 1 + s)

            # weights stream in behind the stats inputs
            w_t = {}
            for nm, d in (("q", wq_d), ("k", wk_d), ("v", wv_d), ("p", wp_d)):
                w_t[nm] = [big.tile([128, C], BF16, name=f"w{nm}{h}")
                           for h in range(2)]
                for h in range(2):
                    nc.sync.dma_start(w_t[nm][h][:],
                                      d[h * 128:(h + 1) * 128, :])

            # f32 x arrives late; only the epilogue residual reads it
            x_t = [big.tile([128, L], F32, name=f"x{h}") for h in range(2)]
            for h in range(2):
                nc.sync.dma_start(x_t[h][:], x_d[h * 128:(h + 1) * 128, :])

            # ------- combine stats -> a (scale), d (shift) per channel ----
            ND = 4 * 5 * 512          # elements covered by the DVE packs
            NT = B * L
            a_t, d_t = [], []
            for h in range(2):
                s2 = sm.tile([128, 2], F32, name=f"s2_{h}")
                nc.vector.bn_aggr(s2[:], s6_dve[h][:])
                sa = sm.tile([128, 1], F32, name=f"sa{h}")
                nc.vector.reduce_sum(sa[:], asum[h][:], axis=mybir.AxisListType.X)
                qa = sm.tile([128, 1], F32, name=f"qa{h}")
                nc.vector.reduce_sum(qa[:], assq[h][:], axis=mybir.AxisListType.X)
                tot = sm.tile([128, 1], F32, name=f"tot{h}")
                nc.vector.scalar_tensor_tensor(
                    out=tot[:], in0=s2[:, 0:1], scalar=float(ND), in1=sa[:],
                    op0=mybir.AluOpType.mult, op1=mybir.AluOpType.add)
                mo2 = sm.tile([128, 1], F32, name=f"mo2{h}")
                nc.vector.tensor_mul(mo2[:], s2[:, 0:1], s2[:, 0:1])
                e2o = sm.tile([128, 1], F32, name=f"e2o{h}")
                nc.vector.tensor_add(e2o[:], s2[:, 1:2], mo2[:])
                totq = sm.tile([128, 1], F32, name=f"totq{h}")
                nc.vector.scalar_tensor_tensor(
                    out=totq[:], in0=e2o[:], scalar=float(ND), in1=qa[:],
                    op0=mybir.AluOpType.mult, op1=mybir.AluOpType.add)
                gm = sm.tile([128, 1], F32, name=f"gm{h}")
                nc.vector.tensor_scalar_mul(gm[:], tot[:], 1.0 / NT)
                ge2 = sm.tile([128, 1], F32, name=f"ge2{h}")
                nc.vector.tensor_scalar_mul(ge2[:], totq[:], 1.0 / NT)
                mm_ = sm.tile([128, 1], F32, name=f"mm{h}")
                nc.vector.tensor_mul(mm_[:], gm[:], gm[:])
                var = sm.tile([128, 1], F32, name=f"var{h}")
                nc.vector.tensor_sub(var[:], ge2[:], mm_[:])
                nc.vector.tensor_scalar_add(var[:], var[:], EPS)
                sd = sm.tile([128, 1], F32, name=f"sd{h}")
                nc.scalar.activation(sd[:], var[:], AF.Sqrt)
                rs = sm.tile([128, 1], F32, name=f"rs{h}")
                nc.vector.reciprocal(rs[:], sd[:])
                a = sm.tile([128, 1], F32, name=f"a{h}")
                nc.vector.tensor_mul(a[:], rs[:], vecs["gam"][h][:])
                ma = sm.tile([128, 1], F32, name=f"ma{h}")
                nc.vector.tensor_mul(ma[:], gm[:], a[:])
                dd = sm.tile([128, 1], F32, name=f"d{h}")
                nc.vector.tensor_sub(dd[:], vecs["bet"][h][:], ma[:])
                a_t.append(a)
                d_t.append(dd)

            # ------- fold BN affine into weights + effective biases -------
            # b*_eff = w @ d + b uses the RAW weights (tiny matvecs), then
            # w is scaled IN PLACE: w[c, o] *= a[c].
            # d as a bf16 [128,1] for the tiny matvecs
            d16 = [sm.tile([128, 1], BF16, name=f"d16_{h}") for h in range(2)]
            for h in range(2):
                nc.vector.tensor_copy(d16[h][:], d_t[h][:])

            def matvec(wtiles, rhs16, name):
                """out[o] = sum_c w[o, c] * rhs[c] as [2][128, 1] sbuf f32"""
                outs = []
                for oh in range(2):
                    ps = ps_s.tile([128, 1], F32, tag="s", name=f"mv_{name}{oh}")
                    for ch in range(2):
                        nc.tensor.matmul(
                            ps[:],
                            wtiles[ch][:, oh * 128:(oh + 1) * 128],
                            rhs16[ch][:],
                            start=(ch == 0), stop=(ch == 1),
                        )
                    o = sm.tile([128, 1], F32, name=f"mvo_{name}{oh}")
                    nc.vector.tensor_copy(o[:], ps[:])
                    outs.append(o)
                return outs

            wqd = matvec(w_t["q"], d16, "q")
            wkd = matvec(w_t["k"], d16, "k")
            wvd = matvec(w_t["v"], d16, "v")
            bq_e, bk_e = [], []
            for oh in range(2):
                t = sm.tile([128, 1], F32, name=f"bqe{oh}")
                nc.vector.tensor_add(t[:], wqd[oh][:], vecs["bq"][oh][:])
                bq_e.append(t)
                t = sm.tile([128, 1], F32, name=f"bke{oh}")
                nc.vector.tensor_add(t[:], wkd[oh][:], vecs["bk"][oh][:])
                bk_e.append(t)
            # bpe_eff = bpe + wp @ (wv @ d)
            wvd16 = [sm.tile([128, 1], BF16, name=f"wvd16_{h}")
                     for h in range(2)]
            for h in range(2):
                nc.vector.tensor_copy(wvd16[h][:], wvd[h][:])
            wpwvd = matvec(w_t["p"], wvd16, "p")
            bp_e = []
            for oh in range(2):
                t = sm.tile([128, 1], F32, name=f"bpe_e{oh}")
                nc.vector.tensor_add(t[:], wpwvd[oh][:], vecs["bpe"][oh][:])
                bp_e.append(t)

            for nm in ("q", "k", "v"):
                for h in range(2):
                    nc.vector.tensor_scalar_mul(
                        w_t[nm][h][:], w_t[nm][h][:], a_t[h][:])

            # ---------------- projections (read x16 directly) -------------
            q_t = [big.tile([128, M], BF16, name=f"q{h}") for h in range(2)]
            k_t = [big.tile([128, L], BF16, name=f"k{h}") for h in range(2)]
            vT_t = big.tile([128, NJT * 256], BF16, name="vT")

            for oh in range(2):
                for it in range(M // 512):
                    ps = ps_s.tile([128, 512], F32, tag="s", name="ps_q")
                    for ch in range(2):
                        nc.tensor.matmul(
                            ps[:],
                            w_t["q"][ch][:, oh * 128:(oh + 1) * 128],
                            x16_t[ch][:, it * 512:(it + 1) * 512],
                            start=(ch == 0), stop=(ch == 1),
                        )
                    nc.vector.tensor_scalar_add(
                        q_t[oh][:, it * 512:(it + 1) * 512], ps[:],
                        bq_e[oh][:])

            for oh in range(2):
                for it in range(L // 512):
                    ps = ps_s.tile([128, 512], F32, tag="s", name="ps_k")
                    for ch in range(2):
                        nc.tensor.matmul(
                            ps[:],
                            w_t["k"][ch][:, oh * 128:(oh + 1) * 128],
                            x16_t[ch][:, it * 512:(it + 1) * 512],
                            start=(ch == 0), stop=(ch == 1),
                        )
                    nc.vector.tensor_scalar_add(
                        k_t[oh][:, it * 512:(it + 1) * 512], ps[:],
                        bk_e[oh][:])

            for lt in range(NJT):
                ps = ps_s.tile([128, 512], F32, tag="s", name="ps_v")
                for ch in range(2):
                    nc.tensor.matmul(
                        ps[:, 0:256],
                        x16_t[ch][:, lt * 128:(lt + 1) * 128],
                        w_t["v"][ch][:],
                        start=(ch == 0), stop=(ch == 1),
                    )
                nc.vector.tensor_copy(
                    vT_t[:, lt * 256:(lt + 1) * 256], ps[:, 0:256])

            ones_t = big.tile([128, 128], BF16, name="ones")
            nc.vector.memset(ones_t[:], 1.0)

            # ---------------- attention, chunk by chunk ----------------
            for cn in range(NCHUNK):
                i0 = cn * CH
                pT = ptp.tile([128, NJT * CH], BF16, tag="pT", name=f"pT{cn}")
                for jp in range(NJT // 2):
                    ps = ps_s.tile([128, 1024], F32, tag="s", name="ps_sc")
                    for half in range(2):
                        jt = jp * 2 + half
                        for ch in range(2):
                            nc.tensor.matmul(
                                ps[:, half * 512:(half + 1) * 512],
                                k_t[ch][:, jt * 128:(jt + 1) * 128],
                                q_t[ch][:, i0:i0 + CH],
                                start=(ch == 0), stop=(ch == 1),
                            )
                    nc.scalar.activation(
                        pT[:, jp * 1024:(jp + 1) * 1024], ps[:],
                        AF.Exp, scale=SCALE)

                ps_av = [ps_acc.tile([128, CH], F32, tag=f"av{ch}",
                                     name=f"av{ch}_{cn}") for ch in range(2)]
                ps_den = ps_acc.tile([128, CH], F32, tag="den",
                                     name=f"den{cn}")
                for jt in range(NJT):
                    pslice = pT[:, jt * CH:(jt + 1) * CH]
                    for ch in range(2):
                        nc.tensor.matmul(
                            ps_av[ch][:],
                            vT_t[:, jt * 256 + ch * 128:jt * 256 + (ch + 1) * 128],
                            pslice,
                            start=(jt == 0), stop=(jt == NJT - 1),
                        )
                    nc.tensor.matmul(
                        ps_den[:], ones_t[:], pslice,
                        start=(jt == 0), stop=(jt == NJT - 1),
                    )

                rec = epi.tile([128, CH], F32, tag="rec", name=f"rec{cn}")
                nc.vector.reciprocal_approx_fast(rec[:], ps_den[:])

                at_t = []
                for ch in range(2):
                    at = epi.tile([128, CH], BF16, tag=f"at{ch}",
                                  name=f"at{ch}_{cn}")
                    nc.vector.tensor_mul(at[:], ps_av[ch][:], rec[:])
                    at_t.append(at)

                for oh in range(2):
                    ps = ps_o.tile([128, CH], F32, tag="o", name=f"po{oh}_{cn}")
                    for ch in range(2):
                        nc.tensor.matmul(
                            ps[:],
                            w_t["p"][ch][:, oh * 128:(oh + 1) * 128],
                            at_t[ch][:],
                            start=(ch == 0), stop=(ch == 1),
                        )
                    res = epi.tile([128, CH], F32, tag="res",
                                   name=f"res{oh}_{cn}")
                    nc.vector.scalar_tensor_tensor(
                        out=res[:], in0=ps[:], scalar=bp_e[oh][:],
                        in1=x_t[oh][:, i0:i0 + CH],
                        op0=mybir.AluOpType.add, op1=mybir.AluOpType.add,
                    )
                    nc.sync.dma_start(
                        out_d[oh * 128:(oh + 1) * 128, i0:i0 + CH], res[:])

    nc.compile()
    return nc


def kernel(x, gamma, beta, wq, bq, wk, bk, wv, bv, wp, bp):
    global _COMPILED, LAST_EXEC_NS
    x = np.asarray(x, np.float32)
    if _COMPILED is None:
        _COMPILED = _build()
    nc = _COMPILED

    common = {
        "wqT": np.ascontiguousarray(np.asarray(wq, np.float32).T).astype(ml_dtypes.bfloat16),
        "wkT": np.ascontiguousarray(np.asarray(wk, np.float32).T).astype(ml_dtypes.bfloat16),
        "wvT": np.ascontiguousarray(np.asarray(wv, np.float32).T).astype(ml_dtypes.bfloat16),
        "wpT": np.ascontiguousarray(np.asarray(wp, np.float32).T).astype(ml_dtypes.bfloat16),
        "bq": np.asarray(bq, np.float32).reshape(C, 1),
        "bk": np.asarray(bk, np.float32).reshape(C, 1),
        "bpe": (np.asarray(bp, np.float32)
                + np.asarray(wp, np.float32) @ np.asarray(bv, np.float32)
                ).reshape(C, 1),
        "gamma": np.asarray(gamma, np.float32).reshape(C, 1),
        "beta": np.asarray(beta, np.float32).reshape(C, 1),
    }

    x16 = [np.ascontiguousarray(x[b]).astype(ml_dtypes.float8_e4m3) for b in range(B)]

    in_maps = []
    for core in range(N_CORES):
        b, qh = core // 2, core % 2
        xb = x[b]
        if qh:
            xb = np.ascontiguousarray(np.roll(xb, -M, axis=1))
        others = np.concatenate([x16[s] for s in range(B) if s != b])
        in_maps.append({"x": xb, "x16": xb.astype(ml_dtypes.bfloat16),
                        "xs": others, **common})

    trace = os.environ.get("BASS_KERNEL_TRACE", "") == "1"
    res = bass_utils.run_bass_kernel_spmd(
        nc, in_maps, core_ids=list(range(N_CORES)), trace=trace)
    LAST_EXEC_NS = res.exec_time_ns

    out = np.empty((B, C, L), np.float32)
    for core in range(N_CORES):
        b, qh = core // 2, core % 2
        out[b, :, qh * M:(qh + 1) * M] = res.results[core]["out"]
    return out
